# revision 5
# baseline (speedup 1.0000x reference)
"""Trainium2 Bass kernel for batched softmax-attention readout:

    out[b] = softmax(S[b], axis=-1) @ U[b]

Shapes (hardcoded): S [B=128, T=2048, J=128] f32, U [B=128, J=128, d=512] f32,
out [B=128, T=2048, d=512] f32.

Sharding: batch dim B split across 8 NeuronCores (16 batches/core), fully
data-parallel (softmax and the A@U matmul are batch-local; no collectives).

Per-core pipeline, per batch b, with T split into 16 chunks of 128 rows
(row t = c*128 + p so every HBM access is contiguous):
  1. DMA S[b] -> SBUF [128p, 16c, 128j]; DMA U[b] -> SBUF [128j, 512d]
  2. ScalarE: E = exp(S)  (no max-subtraction needed: |S| <~ 6 so exp is
     fp32-safe; matches softmax to ~1e-7 rel)
  3. VectorE: r = sum_j E;  rinv = 1/r  ([128p, 16c])
  4. TensorE: per chunk, transpose E chunk -> PSUM [j, t]
  5. ScalarE/VectorE: copy back to SBUF (lhsT layout)
  6. TensorE: matmul(out_psum[t, d] = E_chunk @ U[b]) in float32r
  7. ScalarE/VectorE: out_sbuf = out_psum * rinv[:, c]  (fused normalize +
     mandatory PSUM->SBUF evacuation)
  8. DMA out chunk groups -> HBM (contiguous 256KB*OG blocks)
"""

import sys

sys.path.insert(0, "/opt/trn_rl_repo")

from contextlib import ExitStack

import numpy as np

import concourse.bass as bass
import concourse.mybir as mybir
import concourse.tile as tile
from concourse import bacc
from concourse.bass_utils import run_bass_kernel_spmd
from concourse.masks import make_identity

# Problem shapes
B, T, J, D = 128, 2048, 128, 512
N_CORES = 8
BPC = B // N_CORES  # batches per core
P = 128
C = T // P  # T-chunks per batch

# Tuning knobs
MM_DTYPE = "f32r"  # 'f32r' | 'f32' | 'bf16'
EXP_SPLIT = 4  # activation ops per batch (finer -> earlier transposes)
OG = 4  # out chunks per output DMA (OG*256KB contiguous)
S_SPLIT = 2  # input-S DMAs per batch
OUT_ACT_EVERY = 3  # every k-th out-copyback goes to ScalarE, rest VectorE
ET_ON_ACT = True  # lhsT copyback engine: True=ScalarE, False=VectorE
BUFS = dict(s=2, u=2, et=4, o=3, pst=3, pso=4)

F32 = mybir.dt.float32
F32R = mybir.dt.float32r
BF16 = mybir.dt.bfloat16


def build_nc():
    nc = bacc.Bacc(
        "TRN2", target_bir_lowering=False, debug=False, num_devices=N_CORES
    )
    S = nc.dram_tensor("S", [BPC, T, J], F32, kind="ExternalInput").ap()
    U = nc.dram_tensor("U", [BPC, J, D], F32, kind="ExternalInput").ap()
    O = nc.dram_tensor("O", [BPC, T, D], F32, kind="ExternalOutput").ap()

    mm_dt = {"f32r": F32R, "f32": F32, "bf16": BF16}[MM_DTYPE]

    with tile.TileContext(nc) as tc, ExitStack() as ctx:
        consts = ctx.enter_context(tc.tile_pool(name="consts", bufs=1))
        s_pool = ctx.enter_context(tc.tile_pool(name="s", bufs=BUFS["s"]))
        u_pool = ctx.enter_context(tc.tile_pool(name="u", bufs=BUFS["u"]))
        et_pool = ctx.enter_context(tc.tile_pool(name="et", bufs=BUFS["et"]))
        o_pool = ctx.enter_context(tc.tile_pool(name="o", bufs=BUFS["o"]))
        st_pool = ctx.enter_context(tc.tile_pool(name="stats", bufs=2))
        pst = ctx.enter_context(tc.tile_pool(name="pst", bufs=BUFS["pst"], space="PSUM"))
        pso = ctx.enter_context(tc.tile_pool(name="pso", bufs=BUFS["pso"], space="PSUM"))

        ident = consts.tile([P, P], F32)
        make_identity(nc, ident)

        for b in range(BPC):
            # --- loads ---
            s_sb = s_pool.tile([P, C, J], F32)
            s_src = S[b].rearrange("(c p) j -> p c j", p=P)
            for ss in range(S_SPLIT):
                cs = C // S_SPLIT
                sl = slice(ss * cs, (ss + 1) * cs)
                nc.sync.dma_start(s_sb[:, sl, :], s_src[:, sl, :])
            u_sb = u_pool.tile([P, D], F32)
            nc.sync.dma_start(u_sb[:], U[b])
            if mm_dt != F32:
                u_mm = u_pool.tile([P, D], mm_dt)
                nc.vector.tensor_copy(u_mm[:], u_sb[:])
            else:
                u_mm = u_sb

            # --- exp (in place) ---
            for es in range(EXP_SPLIT):
                cs = C // EXP_SPLIT
                sl = slice(es * cs, (es + 1) * cs)
                nc.scalar.activation(
                    s_sb[:, sl, :], s_sb[:, sl, :], mybir.ActivationFunctionType.Exp
                )

            # --- softmax denominator ---
            r = st_pool.tile([P, C], F32)
            nc.vector.reduce_sum(r[:], s_sb[:], axis=mybir.AxisListType.X)
            rinv = st_pool.tile([P, C], F32)
            nc.vector.reciprocal(rinv[:], r[:])

            o_dst = O[b].rearrange("(c p) d -> p c d", p=P)

            # --- per chunk: transpose -> copyback -> matmul -> scale -> store
            # PE stream is software-pipelined depth 1: transpose(c+1) is
            # emitted before matmul(c) so the lhsT copyback latency hides.
            et_ps = [None] * C
            et_sb = [None] * C
            o_ps = [None] * C
            o_sb = [None] * (C // OG)

            def do_transpose(c):
                et_ps[c] = pst.tile([P, P], F32, tag="et_ps", name=f"et_ps_{b}_{c}")
                nc.tensor.transpose(et_ps[c][:], s_sb[:, c, :], ident[:])
                et_sb[c] = et_pool.tile(
                    [P, P], mm_dt, tag="et_sb", name=f"et_sb_{b}_{c}"
                )
                if ET_ON_ACT:
                    nc.scalar.copy(et_sb[c][:], et_ps[c][:])
                else:
                    nc.vector.tensor_copy(et_sb[c][:], et_ps[c][:])

            def do_matmul(c):
                o_ps[c] = pso.tile([P, D], F32, tag="o_ps", name=f"o_ps_{b}_{c}")
                nc.tensor.matmul(
                    o_ps[c][:], et_sb[c][:], u_mm[:], start=True, stop=True
                )
                g, gi = divmod(c, OG)
                if gi == 0:
                    o_sb[g] = o_pool.tile([P, OG, D], F32, tag="o_sb", name=f"o_sb_{b}_{c}")
                if c % OUT_ACT_EVERY == 0:
                    nc.scalar.mul(o_sb[g][:, gi, :], o_ps[c][:], rinv[:, c : c + 1])
                else:
                    nc.vector.tensor_scalar_mul(
                        o_sb[g][:, gi, :], o_ps[c][:], rinv[:, c : c + 1]
                    )
                if gi == OG - 1:
                    nc.sync.dma_start(
                        o_dst[:, g * OG : (g + 1) * OG, :], o_sb[g][:]
                    )

            do_transpose(0)
            for c in range(1, C):
                do_transpose(c)
                do_matmul(c - 1)
            do_matmul(C - 1)

    nc.compile()
    return nc


_NC_CACHE = None


def _get_nc():
    global _NC_CACHE
    if _NC_CACHE is None:
        _NC_CACHE = build_nc()
    return _NC_CACHE


def make_in_maps(U, S):
    U = np.ascontiguousarray(np.asarray(U, dtype=np.float32))
    S = np.ascontiguousarray(np.asarray(S, dtype=np.float32))
    return [
        {
            "S": S[i * BPC : (i + 1) * BPC],
            "U": U[i * BPC : (i + 1) * BPC],
        }
        for i in range(N_CORES)
    ]


def kernel(U, S):
    nc = _get_nc()
    in_maps = make_in_maps(U, S)
    res = run_bass_kernel_spmd(nc, in_maps, core_ids=list(range(N_CORES)))
    out = np.concatenate([res.results[i]["O"] for i in range(N_CORES)], axis=0)
    return out


# revision 6
# speedup vs baseline: 338.9183x; 338.9183x over previous
"""Trainium2 Bass kernel for batched softmax-attention readout:

    out[b] = softmax(S[b], axis=-1) @ U[b]

Shapes (hardcoded): S [B=128, T=2048, J=128] f32, U [B=128, J=128, d=512] f32,
out [B=128, T=2048, d=512] f32.

Sharding: batch dim B split across 8 NeuronCores (16 batches/core), fully
data-parallel (softmax and the A@U matmul are batch-local; no collectives).

Per-core pipeline, per batch b, with T split into 16 chunks of 128 rows
(row t = c*128 + p so every HBM access is contiguous):
  1. DMA S[b] -> SBUF [128p, 16c, 128j]; DMA U[b] -> SBUF [128j, 512d]
  2. ScalarE: E = exp(S)  (no max-subtraction needed: |S| <~ 6 so exp is
     fp32-safe; matches softmax to ~1e-7 rel)
  3. VectorE: r = sum_j E;  rinv = 1/r  ([128p, 16c])
  4. TensorE: per chunk, transpose E chunk -> PSUM [j, t]
  5. ScalarE/VectorE: copy back to SBUF (lhsT layout)
  6. TensorE: matmul(out_psum[t, d] = E_chunk @ U[b]) in float32r
  7. ScalarE/VectorE: out_sbuf = out_psum * rinv[:, c]  (fused normalize +
     mandatory PSUM->SBUF evacuation)
  8. DMA out chunk groups -> HBM (contiguous 256KB*OG blocks)
"""

import sys

sys.path.insert(0, "/opt/trn_rl_repo")

from contextlib import ExitStack

import numpy as np

import concourse.bass as bass
import concourse.mybir as mybir
import concourse.tile as tile
from concourse import bacc
from concourse.bass_utils import run_bass_kernel_spmd
from concourse.masks import make_identity

# Problem shapes
B, T, J, D = 128, 2048, 128, 512
N_CORES = 8
BPC = B // N_CORES  # batches per core
P = 128
C = T // P  # T-chunks per batch

# Tuning knobs
MM_DTYPE = "f32r"  # 'f32r' | 'f32' | 'bf16'
EXP_SPLIT = 4  # activation ops per batch (finer -> earlier transposes)
OG = 4  # out chunks per output DMA (OG*256KB contiguous)
S_SPLIT = 2  # input-S DMAs per batch
OUT_ACT_EVERY = 3  # every k-th out-copyback goes to ScalarE, rest VectorE
ET_ON_ACT = True  # lhsT copyback engine: True=ScalarE, False=VectorE
BUFS = dict(s=2, u=2, et=4, o=3, pst=3, pso=4)

F32 = mybir.dt.float32
F32R = mybir.dt.float32r
BF16 = mybir.dt.bfloat16


def build_nc(repeat=1):
    nc = bacc.Bacc(
        "TRN2", target_bir_lowering=False, debug=False, num_devices=N_CORES
    )
    S = nc.dram_tensor("S", [BPC, T, J], F32, kind="ExternalInput").ap()
    U = nc.dram_tensor("U", [BPC, J, D], F32, kind="ExternalInput").ap()
    O = nc.dram_tensor("O", [BPC, T, D], F32, kind="ExternalOutput").ap()

    mm_dt = {"f32r": F32R, "f32": F32, "bf16": BF16}[MM_DTYPE]

    with tile.TileContext(nc) as tc, ExitStack() as ctx:
        consts = ctx.enter_context(tc.tile_pool(name="consts", bufs=1))
        s_pool = ctx.enter_context(tc.tile_pool(name="s", bufs=BUFS["s"]))
        u_pool = ctx.enter_context(tc.tile_pool(name="u", bufs=BUFS["u"]))
        et_pool = ctx.enter_context(tc.tile_pool(name="et", bufs=BUFS["et"]))
        o_pool = ctx.enter_context(tc.tile_pool(name="o", bufs=BUFS["o"]))
        st_pool = ctx.enter_context(tc.tile_pool(name="stats", bufs=2))
        pst = ctx.enter_context(tc.tile_pool(name="pst", bufs=BUFS["pst"], space="PSUM"))
        pso = ctx.enter_context(tc.tile_pool(name="pso", bufs=BUFS["pso"], space="PSUM"))

        ident = consts.tile([P, P], F32)
        make_identity(nc, ident)

        loop_ctx = tc.For_i(0, repeat, 1) if repeat > 1 else None
        if loop_ctx is not None:
            ctx.enter_context(loop_ctx)

        for b in range(BPC):
            # --- loads ---
            s_sb = s_pool.tile([P, C, J], F32)
            s_src = S[b].rearrange("(c p) j -> p c j", p=P)
            for ss in range(S_SPLIT):
                cs = C // S_SPLIT
                sl = slice(ss * cs, (ss + 1) * cs)
                nc.sync.dma_start(s_sb[:, sl, :], s_src[:, sl, :])
            u_sb = u_pool.tile([P, D], F32)
            nc.sync.dma_start(u_sb[:], U[b])
            if mm_dt != F32:
                u_mm = u_pool.tile([P, D], mm_dt)
                nc.vector.tensor_copy(u_mm[:], u_sb[:])
            else:
                u_mm = u_sb

            # --- exp (in place) ---
            for es in range(EXP_SPLIT):
                cs = C // EXP_SPLIT
                sl = slice(es * cs, (es + 1) * cs)
                nc.scalar.activation(
                    s_sb[:, sl, :], s_sb[:, sl, :], mybir.ActivationFunctionType.Exp
                )

            # --- softmax denominator ---
            r = st_pool.tile([P, C], F32)
            nc.vector.reduce_sum(r[:], s_sb[:], axis=mybir.AxisListType.X)
            rinv = st_pool.tile([P, C], F32)
            nc.vector.reciprocal(rinv[:], r[:])

            o_dst = O[b].rearrange("(c p) d -> p c d", p=P)

            # --- per chunk: transpose -> copyback -> matmul -> scale -> store
            # PE stream is software-pipelined depth 1: transpose(c+1) is
            # emitted before matmul(c) so the lhsT copyback latency hides.
            et_ps = [None] * C
            et_sb = [None] * C
            o_ps = [None] * C
            o_sb = [None] * (C // OG)

            def do_transpose(c):
                et_ps[c] = pst.tile([P, P], F32, tag="et_ps", name=f"et_ps_{b}_{c}")
                nc.tensor.transpose(et_ps[c][:], s_sb[:, c, :], ident[:])
                et_sb[c] = et_pool.tile(
                    [P, P], mm_dt, tag="et_sb", name=f"et_sb_{b}_{c}"
                )
                if ET_ON_ACT:
                    nc.scalar.copy(et_sb[c][:], et_ps[c][:])
                else:
                    nc.vector.tensor_copy(et_sb[c][:], et_ps[c][:])

            def do_matmul(c):
                o_ps[c] = pso.tile([P, D], F32, tag="o_ps", name=f"o_ps_{b}_{c}")
                nc.tensor.matmul(
                    o_ps[c][:], et_sb[c][:], u_mm[:], start=True, stop=True
                )
                g, gi = divmod(c, OG)
                if gi == 0:
                    o_sb[g] = o_pool.tile([P, OG, D], F32, tag="o_sb", name=f"o_sb_{b}_{c}")
                if c % OUT_ACT_EVERY == 0:
                    nc.scalar.mul(o_sb[g][:, gi, :], o_ps[c][:], rinv[:, c : c + 1])
                else:
                    nc.vector.tensor_scalar_mul(
                        o_sb[g][:, gi, :], o_ps[c][:], rinv[:, c : c + 1]
                    )
                if gi == OG - 1:
                    nc.sync.dma_start(
                        o_dst[:, g * OG : (g + 1) * OG, :], o_sb[g][:]
                    )

            do_transpose(0)
            for c in range(1, C):
                do_transpose(c)
                do_matmul(c - 1)
            do_matmul(C - 1)

    nc.compile()
    return nc


_NC_CACHE = None


def _get_nc():
    global _NC_CACHE
    if _NC_CACHE is None:
        _NC_CACHE = build_nc()
    return _NC_CACHE


def make_in_maps(U, S):
    U = np.ascontiguousarray(np.asarray(U, dtype=np.float32))
    S = np.ascontiguousarray(np.asarray(S, dtype=np.float32))
    return [
        {
            "S": S[i * BPC : (i + 1) * BPC],
            "U": U[i * BPC : (i + 1) * BPC],
        }
        for i in range(N_CORES)
    ]


def kernel(U, S):
    nc = _get_nc()
    in_maps = make_in_maps(U, S)
    res = run_bass_kernel_spmd(nc, in_maps, core_ids=list(range(N_CORES)))
    out = np.concatenate([res.results[i]["O"] for i in range(N_CORES)], axis=0)
    return out


# revision 12
# speedup vs baseline: 459.1952x; 1.3549x over previous
"""Trainium2 Bass kernel for batched softmax-attention readout:

    out[b] = softmax(S[b], axis=-1) @ U[b]

Shapes (hardcoded): S [B=128, T=2048, J=128] f32, U [B=128, J=128, d=512] f32,
out [B=128, T=2048, d=512] f32.

Sharding: batch dim B split across 8 NeuronCores (16 batches/core), fully
data-parallel (softmax and the A@U matmul are batch-local; no collectives).

Per-core pipeline, per batch b, with T split into 16 chunks of 128 rows
(row t = c*128 + p so every HBM access is contiguous):
  1. DMA S[b] -> SBUF [128p, 16c, 128j]; DMA U[b] -> SBUF [128j, 512d]
  2. ScalarE: E = exp(S)  (no max-subtraction needed: |S| <~ 6 so exp is
     fp32-safe; matches softmax to ~1e-7 rel)
  3. VectorE: r = sum_j E;  rinv = 1/r  ([128p, 16c])
  4. TensorE: per chunk, transpose E chunk -> PSUM [j, t]
  5. ScalarE/VectorE: copy back to SBUF (lhsT layout)
  6. TensorE: matmul(out_psum[t, d] = E_chunk @ U[b]) in float32r
  7. ScalarE/VectorE: out_sbuf = out_psum * rinv[:, c]  (fused normalize +
     mandatory PSUM->SBUF evacuation)
  8. DMA out chunk groups -> HBM (contiguous 256KB*OG blocks)
"""

import sys

sys.path.insert(0, "/opt/trn_rl_repo")

from contextlib import ExitStack

import numpy as np

import concourse.bass as bass
import concourse.mybir as mybir
import concourse.tile as tile
from concourse import bacc
from concourse.bass_utils import run_bass_kernel_spmd
from concourse.masks import make_identity

# Problem shapes
B, T, J, D = 128, 2048, 128, 512
N_CORES = 8
BPC = B // N_CORES  # batches per core
P = 128
C = T // P  # T-chunks per batch

# Tuning knobs
MM_DTYPE = "f32r"  # 'f32r' | 'f32' | 'bf16'
EXP_SPLIT = 4  # activation ops per batch (finer -> earlier transposes)
OG = 4  # out chunks per output DMA (OG*256KB contiguous)
S_SPLIT = 2  # input-S DMAs per batch
OUT_ACT_EVERY = 3  # every k-th out-copyback goes to ScalarE, rest VectorE
ET_ON_ACT = True  # lhsT copyback engine: True=ScalarE, False=VectorE
BUFS = dict(s=3, u=2, et=3, o=4, pst=2, pso=5)

F32 = mybir.dt.float32
F32R = mybir.dt.float32r
BF16 = mybir.dt.bfloat16


def build_nc(repeat=1, mm_dtype=None, exp_split=None, og=None, s_split=None,
             out_act_every=None, et_on_act=None, bufs=None, skip_out_dma=False,
             skip_in_dma=False, in_dma_gpsimd=False, tg=4):
    mm_dtype = MM_DTYPE if mm_dtype is None else mm_dtype
    exp_split = EXP_SPLIT if exp_split is None else exp_split
    og = OG if og is None else og
    s_split = S_SPLIT if s_split is None else s_split
    out_act_every = OUT_ACT_EVERY if out_act_every is None else out_act_every
    et_on_act = ET_ON_ACT if et_on_act is None else et_on_act
    bufs = dict(BUFS, **(bufs or {}))
    nc = bacc.Bacc(
        "TRN2", target_bir_lowering=False, debug=False, num_devices=N_CORES
    )
    S = nc.dram_tensor("S", [BPC, T, J], F32, kind="ExternalInput").ap()
    U = nc.dram_tensor("U", [BPC, J, D], F32, kind="ExternalInput").ap()
    O = nc.dram_tensor("O", [BPC, T, D], F32, kind="ExternalOutput").ap()

    mm_dt = {"f32r": F32R, "f32": F32, "bf16": BF16}[mm_dtype]

    with tile.TileContext(nc) as tc, ExitStack() as ctx:
        consts = ctx.enter_context(tc.tile_pool(name="consts", bufs=1))
        s_pool = ctx.enter_context(tc.tile_pool(name="s", bufs=bufs["s"]))
        u_pool = ctx.enter_context(tc.tile_pool(name="u", bufs=bufs["u"]))
        et_pool = ctx.enter_context(tc.tile_pool(name="et", bufs=bufs["et"]))
        o_pool = ctx.enter_context(tc.tile_pool(name="o", bufs=bufs["o"]))
        st_pool = ctx.enter_context(tc.tile_pool(name="stats", bufs=2))
        pst = ctx.enter_context(tc.tile_pool(name="pst", bufs=bufs["pst"], space="PSUM"))
        pso = ctx.enter_context(tc.tile_pool(name="pso", bufs=bufs["pso"], space="PSUM"))

        ident = consts.tile([P, P], F32)
        make_identity(nc, ident)

        loop_ctx = tc.For_i(0, repeat, 1) if repeat > 1 else None
        if loop_ctx is not None:
            ctx.enter_context(loop_ctx)

        for b in range(BPC):
            # --- loads ---
            s_sb = s_pool.tile([P, C, J], F32)
            s_src = S[b].rearrange("(c p) j -> p c j", p=P)
            for ss in range(s_split):
                cs = C // s_split
                sl = slice(ss * cs, (ss + 1) * cs)
                if not skip_in_dma:
                    eng = nc.gpsimd if in_dma_gpsimd else nc.sync
                    eng.dma_start(s_sb[:, sl, :], s_src[:, sl, :])
            if skip_in_dma:
                nc.vector.memset(s_sb[:, 0:1, :], 0.1)
            u_sb = u_pool.tile([P, D], F32)
            if not skip_in_dma:
                (nc.gpsimd if in_dma_gpsimd else nc.sync).dma_start(u_sb[:], U[b])
            if mm_dt != F32:
                u_mm = u_pool.tile([P, D], mm_dt)
                nc.vector.tensor_copy(u_mm[:], u_sb[:])
            else:
                u_mm = u_sb

            # --- exp (in place) ---
            for es in range(exp_split):
                cs = C // exp_split
                sl = slice(es * cs, (es + 1) * cs)
                nc.scalar.activation(
                    s_sb[:, sl, :], s_sb[:, sl, :], mybir.ActivationFunctionType.Exp
                )

            # --- softmax denominator ---
            r = st_pool.tile([P, C], F32)
            nc.vector.reduce_sum(r[:], s_sb[:], axis=mybir.AxisListType.X)
            rinv = st_pool.tile([P, C], F32)
            nc.vector.reciprocal(rinv[:], r[:])

            o_dst = O[b].rearrange("(c p) d -> p c d", p=P)

            # --- per chunk-group of TG=4: 4 transposes into one PSUM bank,
            # ONE merged lhsT copyback, then 4 matmuls + scaled evacuations.
            # PE stream is software-pipelined one group deep so the copyback
            # latency hides behind the next group's transposes.
            TG = tg
            NG = C // TG
            et_sb = [None] * NG
            o_sb = [None] * (C // og)

            def do_transposes(g):
                et_ps = pst.tile([P, TG, P], F32, tag="et_ps", name=f"et_ps_{b}_{g}")
                for k in range(TG):
                    nc.tensor.transpose(
                        et_ps[:, k, :], s_sb[:, g * TG + k, :], ident[:]
                    )
                et_sb[g] = et_pool.tile(
                    [P, TG, P], mm_dt, tag="et_sb", name=f"et_sb_{b}_{g}"
                )
                if (g % 2 == 0) if et_on_act else False:
                    nc.scalar.copy(et_sb[g][:], et_ps[:])
                else:
                    nc.vector.tensor_copy(et_sb[g][:], et_ps[:])

            def do_matmul(c):
                o_ps = pso.tile([P, D], F32, tag="o_ps", name=f"o_ps_{b}_{c}")
                g, k = divmod(c, TG)
                nc.tensor.matmul(
                    o_ps[:], et_sb[g][:, k, :], u_mm[:], start=True, stop=True
                )
                og_g, gi = divmod(c, og)
                if gi == 0:
                    o_sb[og_g] = o_pool.tile(
                        [P, og, D], F32, tag="o_sb", name=f"o_sb_{b}_{c}"
                    )
                if c % out_act_every == 0:
                    nc.scalar.mul(o_sb[og_g][:, gi, :], o_ps[:], rinv[:, c : c + 1])
                else:
                    nc.vector.tensor_scalar_mul(
                        o_sb[og_g][:, gi, :], o_ps[:], rinv[:, c : c + 1]
                    )
                if gi == og - 1 and not skip_out_dma:
                    nc.sync.dma_start(
                        o_dst[:, og_g * og : (og_g + 1) * og, :], o_sb[og_g][:]
                    )

            do_transposes(0)
            for g in range(1, NG):
                do_transposes(g)
                for k in range(TG):
                    do_matmul((g - 1) * TG + k)
            for k in range(TG):
                do_matmul((NG - 1) * TG + k)

    nc.compile()
    return nc


_NC_CACHE = None


def _get_nc():
    global _NC_CACHE
    if _NC_CACHE is None:
        _NC_CACHE = build_nc()
    return _NC_CACHE


def make_in_maps(U, S):
    U = np.ascontiguousarray(np.asarray(U, dtype=np.float32))
    S = np.ascontiguousarray(np.asarray(S, dtype=np.float32))
    return [
        {
            "S": S[i * BPC : (i + 1) * BPC],
            "U": U[i * BPC : (i + 1) * BPC],
        }
        for i in range(N_CORES)
    ]


def kernel(U, S):
    nc = _get_nc()
    in_maps = make_in_maps(U, S)
    try:
        res = run_bass_kernel_spmd(nc, in_maps, core_ids=list(range(N_CORES)))
    except Exception:
        # transient device/runtime hiccup: retry once
        res = run_bass_kernel_spmd(nc, in_maps, core_ids=list(range(N_CORES)))
    out = np.concatenate([res.results[i]["O"] for i in range(N_CORES)], axis=0)
    return out
